# revision 9
# baseline (speedup 1.0000x reference)
"""EvoformerPermuter Trainium2 kernel, v3 (group-pipelined phases).

Same math as v2 (block-stationary Sinkhorn, bf16 stationaries, T=5),
with setup / Sinkhorn / final pipelined across two batch groups and a
unified 4-slot PSUM rotation.
"""
import numpy as np
import ml_dtypes
from contextlib import ExitStack

import concourse.bacc as bacc
import concourse.tile as tile
import concourse.mybir as mybir
from concourse.masks import make_identity
from concourse.bass_utils import run_bass_kernel_spmd

F32 = mybir.dt.float32
F32R = mybir.dt.float32r
BF16 = mybir.dt.bfloat16
EXP = mybir.ActivationFunctionType.Exp

B, N, D, EDIM = 64, 512, 256, 128
NCORES = 8
NB = B // NCORES          # batches per core
C = N // 128              # chunks per matrix dim
DC = D // 128             # d-dim chunks
T_ITERS = 5
GRPS = [(0, 6), (6, 2)]   # (first batch, size) sinkhorn groups
NG = len(GRPS)

_CACHE = {}


def _build():
    nc = bacc.Bacc()
    xt_d = nc.dram_tensor("xt", [NB, 128, 2, DC, N], BF16, kind="ExternalInput")
    wab_d = nc.dram_tensor("wab", [128, DC, 2, EDIM], BF16, kind="ExternalInput")
    out_d = nc.dram_tensor("out", [NB, 128, C, N], BF16, kind="ExternalOutput")

    with tile.TileContext(nc) as tc, ExitStack() as ctx:
        ctx.enter_context(nc.allow_low_precision(
            reason="bf16 stationaries/IO: 8.6e-3 end-to-end vs 2e-2 budget"))
        res = ctx.enter_context(tc.tile_pool(name="res", bufs=1))
        ps = ctx.enter_context(tc.tile_pool(name="ps", bufs=3, space="PSUM"))
        mvp = ctx.enter_context(tc.tile_pool(name="mvp", bufs=2, space="PSUM"))
        sx = ctx.enter_context(tc.tile_pool(name="sx", bufs=3))
        sy = ctx.enter_context(tc.tile_pool(name="sy", bufs=2))
        wp = ctx.enter_context(tc.tile_pool(name="wp", bufs=2))
        mp = ctx.enter_context(tc.tile_pool(name="mp", bufs=2))
        fsb = ctx.enter_context(tc.tile_pool(name="fsb", bufs=6))

        ident = res.tile([128, 128], F32)
        make_identity(nc, ident)
        identB = res.tile([128, 128], BF16)
        make_identity(nc, identB)

        wab = res.tile([128, DC, 2, EDIM], BF16)
        ones1 = res.tile([128, 1], BF16)
        nc.vector.memset(ones1, 1.0)

        sb_E = res.tile([128, NB, C, N], BF16)   # i on partitions
        sb_ET = res.tile([128, NB, C, N], BF16)  # j on partitions
        sb_P = res.tile([128, NB, C, N], BF16)   # output staging
        invd1W = res.tile([128, C * NB], F32)    # cols c*NB + b
        invd2W = res.tile([128, C * NB], F32)
        inv1b = invd1W.rearrange("p (c b) -> p c b", b=NB)
        inv2b = invd2W.rearrange("p (c b) -> p c b", b=NB)
        Wu32 = [res.tile([128, C * gb * 2], BF16, name=f"Wu32_{g}")
                for g, (b0, gb) in enumerate(GRPS)]
        Wv32 = [res.tile([128, C * gb * 2], BF16, name=f"Wv32_{g}")
                for g, (b0, gb) in enumerate(GRPS)]
        Fs = [res.tile([gb * 2, 2, N], BF16, name=f"Fs_{g}")
              for g, (b0, gb) in enumerate(GRPS)]

        def setup_stage1(b):
            xt = sx.tile([128, 2, DC, N], BF16, tag="xt", name="xt")
            if b <= 2:
                nc.sync.dma_start(xt[:, 0], xt_d[b][:, 0])
                if b == 0:
                    nc.sync.dma_start(wab, wab_d[:, :])
                nc.sync.dma_start(xt[:, 1], xt_d[b][:, 1])
            else:
                nc.sync.dma_start(xt, xt_d[b])

            pab_t = ps.tile([128, 2, N], F32, tag="ps", name="pab")
            for w in range(2):
                for dc in range(DC):
                    nc.tensor.matmul(pab_t[:, w, :],
                                     wab[:, dc, w, :], xt[:, w, dc, :],
                                     start=(dc == 0), stop=(dc == DC - 1))
            ab = sy.tile([128, 2, N], BF16, tag="ab", name="ab")
            nc.vector.tensor_copy(ab[:, 0, :], pab_t[:, 0, :])
            nc.vector.tensor_copy(ab[:, 1, :], pab_t[:, 1, :])
            return ab

        def setup_stage2(b, ab, pump=None):
            def _p():
                if pump is not None:
                    pump()
            # ab[:, 0] = bT (Wb^T xi^T), ab[:, 1] = aT (Wa_s^T xo^T)
            for h in range(2):          # aff/exp in [128, 1024] halves
                pa = ps.tile([128, 2, N], F32, tag="ps", name="pa")
                for c2 in range(2):
                    ci = 2 * h + c2
                    nc.tensor.matmul(pa[:, c2, :],
                                     ab[:, 1, 128 * ci:128 * (ci + 1)],
                                     ab[:, 0, :], start=True, stop=True)
                nc.scalar.activation(sb_E[:, b, 2 * h:2 * h + 2, :], pa, EXP)
            _p()
            for h in range(2):
                pb = ps.tile([128, 2, N], F32, tag="ps", name="pb")
                for c2 in range(2):
                    cj = 2 * h + c2
                    nc.tensor.matmul(pb[:, c2, :],
                                     ab[:, 0, 128 * cj:128 * (cj + 1)],
                                     ab[:, 1, :], start=True, stop=True)
                nc.scalar.activation(sb_ET[:, b, 2 * h:2 * h + 2, :], pb, EXP)
            _p()

            # d1 = colsums(E) via tiny ones-matvecs
            pd_t = ps.tile([128, 2, N], F32, tag="ps", name="pd")
            psD = pd_t.rearrange("p c n -> p (c n)")[:, 0:C]
            for cj in range(C):
                for ci in range(C):
                    nc.tensor.matmul(psD[:, cj:cj + 1],
                                     sb_E[:, b, ci, 128 * cj:128 * (cj + 1)],
                                     ones1, start=(ci == 0), stop=(ci == C - 1))
            nc.vector.reciprocal(
                invd1W.rearrange("p (c b) -> p c b", b=NB)[:, :, b], psD)
            _p()

        def sinkhorn_gen(g):
            b0, gb = GRPS[g]
            inv1 = inv1b[:, :, b0:b0 + gb]
            inv2 = inv2b[:, :, b0:b0 + gb]
            w0 = wp.tile([128, C * gb * 2], BF16, tag=f"W{g}", name=f"w0_{g}")
            nc.vector.memset(w0, 1.0)
            nc.vector.tensor_copy(
                w0.rearrange("p (c b k) -> p c b k", b=gb, k=2)[:, :, :, 0], inv1)
            w_cur = w0
            for t in range(T_ITERS):
                for half in range(2):
                    stat = sb_ET if half == 0 else sb_E
                    last = t == T_ITERS - 1
                    psW = mvp.tile([128, C * gb * 2], F32, tag="mv",
                                   name=f"psW{g}")
                    for bl in range(gb):
                        b = b0 + bl
                        for co in range(C):
                            o = psW[:, (co * gb + bl) * 2:(co * gb + bl) * 2 + 2]
                            for ci in range(C):
                                nc.tensor.matmul(
                                    o, stat[:, b, ci, 128 * co:128 * (co + 1)],
                                    w_cur[:, (ci * gb + bl) * 2:(ci * gb + bl) * 2 + 2],
                                    start=(ci == 0), stop=(ci == C - 1))
                    pv = psW.rearrange("p (c b k) -> p c b k", b=gb, k=2)
                    c0, c1 = pv[:, :, :, 0], pv[:, :, :, 1]
                    invd = inv2 if half == 0 else inv1
                    s_t = mp.tile([128, C * gb], F32, tag=f"s{g}", name=f"s_{g}")
                    s = s_t.rearrange("p (c b) -> p c b", b=gb)
                    if t == 0 and half == 0:
                        nc.vector.reciprocal(inv2, c1)
                        nc.vector.tensor_scalar_add(s, c0, 1.0)
                    else:
                        tmp_t = mp.tile([128, C * gb], F32, tag=f"t{g}",
                                        name=f"tmp_{g}")
                        tmp = tmp_t.rearrange("p (c b) -> p c b", b=gb)
                        nc.vector.tensor_mul(tmp, c1, invd)
                        nc.vector.tensor_add(s, tmp, c0)
                    if not last:
                        w_n = wp.tile([128, C * gb * 2], BF16, tag=f"W{g}",
                                      name=f"wn_{g}")
                        wv = w_n.rearrange("p (c b k) -> p c b k", b=gb, k=2)
                        nc.vector.reciprocal(wv[:, :, :, 1], s)
                        nc.vector.tensor_mul(wv[:, :, :, 0], wv[:, :, :, 1], invd)
                        w_cur = w_n
                    elif half == 0:
                        uv = Wu32[g].rearrange("p (c b k) -> p c b k", b=gb, k=2)
                        nc.vector.reciprocal(uv[:, :, :, 1], s)
                        nc.vector.tensor_mul(uv[:, :, :, 0], uv[:, :, :, 1], invd)
                        w_n = wp.tile([128, C * gb * 2], BF16, tag=f"W{g}",
                                      name=f"wl_{g}")
                        nc.vector.tensor_copy(w_n, Wu32[g])
                        w_cur = w_n
                    else:
                        # final v: cols swapped -> (k=0: v, k=1: v/d1)
                        vv = Wv32[g].rearrange("p (c b k) -> p c b k", b=gb, k=2)
                        nc.vector.reciprocal(vv[:, :, :, 0], s)
                        nc.vector.tensor_mul(vv[:, :, :, 1], vv[:, :, :, 0], invd)
                    yield
            # row-form u/v + per-batch fuv shift, emitted with the last step
            psFu = ps.tile([gb * 2, N], BF16, tag="ps", name=f"psFu{g}")
            psFv = ps.tile([gb * 2, N], BF16, tag="ps", name=f"psFv{g}")
            for ci in range(C):
                nc.tensor.transpose(psFu[:, 128 * ci:128 * (ci + 1)],
                                    Wu32[g][:, ci * gb * 2:(ci + 1) * gb * 2],
                                    identB)
                nc.tensor.transpose(psFv[:, 128 * ci:128 * (ci + 1)],
                                    Wv32[g][:, ci * gb * 2:(ci + 1) * gb * 2],
                                    identB)
            nc.vector.tensor_copy(Fs[g][:, 0, :], psFu)
            nc.scalar.copy(Fs[g][:, 1, :], psFv)
            for bl in range(gb):
                fuv = fsb.tile([2, 2, N], BF16, tag="fuv", name="fuv")
                nc.scalar.dma_start(fuv, Fs[g][2 * bl:2 * bl + 2])
                fuvs[g].append(fuv)

        def final_gen(g, act_assist=False):
            b0, gb = GRPS[g]
            for bl in range(gb):
                b = b0 + bl
                fuv = fuvs[g][bl]
                for h in range(2):
                    psG = ps.tile([128, 2, N], F32, tag="ps", name="psG")
                    for c2 in range(2):
                        ci = 2 * h + c2
                        nc.tensor.matmul(psG[:, c2, :],
                                         fuv[:, 0, 128 * ci:128 * (ci + 1)],
                                         fuv[:, 1, :], start=True, stop=True)
                    if act_assist and h == 0:
                        pg = fsb.tile([128, 2, N], BF16, tag="pg", name="pg")
                        nc.scalar.copy(pg, psG)
                        nc.vector.tensor_mul(sb_P[:, b, 2 * h:2 * h + 2, :],
                                             sb_E[:, b, 2 * h:2 * h + 2, :], pg)
                    else:
                        nc.vector.tensor_mul(sb_P[:, b, 2 * h:2 * h + 2, :],
                                             sb_E[:, b, 2 * h:2 * h + 2, :], psG)
                    if b >= NB - 4:
                        nc.sync.dma_start(out_d[b][:, 2 * h:2 * h + 2, :],
                                          sb_P[:, b, 2 * h:2 * h + 2, :])
                    yield
                if b < NB - 4:
                    nc.sync.dma_start(out_d[b], sb_P[:, b])

        # ---------------- pipelined schedule ----------------
        fuvs = [[] for _ in range(NG)]
        sk = [sinkhorn_gen(g) for g in range(NG)]
        fin = [final_gen(g, act_assist=True) for g in range(NG)]
        # master queue for setup pump spots: finish g0 sinkhorn, then fin0
        from collections import deque
        master = deque([sk[0]] * (2 * T_ITERS + 1)
                       + [fin[0]] * (2 * GRPS[0][1] + 1))

        def pump():
            if master:
                next(master.popleft(), None)

        # PE p-state warmup: ~3us of dummy matmuls while input DMAs fly
        warm = mvp.tile([128, N], F32, tag="mv", name="warm")
        for _ in range(6):
            nc.tensor.matmul(warm[:, 0:128], ident, ident, start=True, stop=True)

        abs_ = {}
        for b in range(NB + 1):
            if b < NB:
                abs_[b] = setup_stage1(b)
            if b >= 1:
                setup_stage2(b - 1, abs_.pop(b - 1),
                             pump=pump if b - 1 >= GRPS[0][1] else None)
        # drain: two concurrent tail chains, leftover master work interleaved
        for i in range(2 * T_ITERS + 1):
            for g in range(1, NG):
                next(sk[g], None)
            if master:
                next(master.popleft(), None)
        while master:
            next(master.popleft(), None)
        for k in range(2 * max(gb for _, gb in GRPS[1:]) + 1):
            for g in range(1, NG):
                next(fin[g], None)

    nc.finalize()
    return nc


def kernel(node_embeddings_inputs, node_masks_inputs, node_embeddings_outputs,
           node_padding_features, positional_encoding_outputs,
           W_a, W_b, w_aff, b_aff):
    # b_aff is a constant bias on aff; softmax(x + const) == softmax(x) along
    # both axes, so it cancels exactly and is ignored.
    bf16 = ml_dtypes.bfloat16
    x_in = np.asarray(node_embeddings_inputs, dtype=np.float32)
    x_out = np.asarray(node_embeddings_outputs, dtype=np.float32)
    mask = np.asarray(node_masks_inputs)
    pad_f = np.asarray(node_padding_features, dtype=np.float32).reshape(1, 1, D)
    pos = np.asarray(positional_encoding_outputs, dtype=np.float32).reshape(1, N, D)

    xi = np.where(mask[..., None], pad_f, x_in)          # [B, N, D]
    xo = x_out + pos
    xiT = xi.reshape(B, N, DC, 128).transpose(0, 3, 2, 1)
    xoT = xo.reshape(B, N, DC, 128).transpose(0, 3, 2, 1)
    xt = np.ascontiguousarray(np.stack([xiT, xoT], axis=2)).astype(bf16)

    wa_s = (np.asarray(W_a, dtype=np.float32)
            * np.asarray(w_aff, dtype=np.float32)[None, :])
    wb_f = np.asarray(W_b, dtype=np.float32)
    wstack = np.stack([wb_f, wa_s], axis=1)              # [D, 2, E]
    wab = np.ascontiguousarray(
        wstack.reshape(DC, 128, 2, EDIM).transpose(1, 0, 2, 3)).astype(bf16)

    if "nc" not in _CACHE:
        _CACHE["nc"] = _build()
    nc = _CACHE["nc"]

    in_maps = []
    for core in range(NCORES):
        sl = slice(core * NB, (core + 1) * NB)
        in_maps.append(dict(xt=xt[sl], wab=wab))
    res = run_bass_kernel_spmd(nc, in_maps, list(range(NCORES)))
    outs = []
    for r in res.results:
        o = np.asarray(r["out"]).astype(np.float32)      # [NB, 128, C, N]
        outs.append(o.transpose(0, 2, 1, 3).reshape(NB, N, N))
    return np.concatenate(outs, axis=0)
